# revision 15
# baseline (speedup 1.0000x reference)
"""Block-sparse linear y = x @ W^T + b on 8 TRN2 NeuronCores.

Problem shape (hardcoded): x [8192, 4096] f32, weight [1024, 64, 64] f32
(64x64 blocks), bias [4096] f32, row_idx/col_idx [1024] int32 over a 64x64
block grid.

Strategy: data-parallel over tokens (1024/core). Each core computes
y^T[feat, tok] = W x^T + b via 64x64 block matmuls packed 4-wide into the
PE-array quadrants with tile_position; compute in bf16 (fp32 PSUM
accumulate).

v2 over the baseline:
- A 32/32 column-block split L/H (chosen by local search so every row has
  ~8 blocks per side) fixes each block's array row-group: side L -> rows
  0-63, side H -> rows 64-127. x^T is packed tile-major on host (tile i =
  [x_{l_i}; x_{h_i}]), so x is DMA'd ONCE (8.4MB instead of the baseline's
  16.8MB double copy) and all x DMAs are issued upfront in first-use order.
- Row pairs are matched so each pair has ~16 blocks per side, and slot
  assignment balances the 4 PE quadrants globally.
- Weight groups stream on the scalar engine's DMA queue (x owns the sync
  queue) with 4-group lookahead.
- Output is stored bf16 in pair-major rows (one [128, 512] DMA per
  eviction); eviction sums the two PSUM banks on VectorE, then ScalarE
  applies bias and casts to bf16. Host un-permutes/transposes/upcasts.

TRN2 PSUM rule (measured): concurrent matmuls that share an output
col-group must write different PSUM banks -> row-group ki writes psum
tile[ki].
"""

from contextlib import ExitStack
from dataclasses import dataclass, field

import numpy as np
import ml_dtypes

import concourse.tile as tile
from concourse import bacc, mybir
from concourse.bass_utils import run_bass_kernel_spmd

BLK = 64
OUT_BLK = 64
IN_BLK = 64
D_IN = IN_BLK * BLK    # 4096
D_OUT = OUT_BLK * BLK  # 4096
N_CORES = 8
WGRP = 16              # weight tiles per DMA group
XCH = 4                # x tiles (128-row groups) per DMA chunk
W_LOOKAHEAD = 4        # weight groups prefetched ahead
N_WARM = 8             # dummy matmuls to warm the HAM clock before real MMs
BF16 = ml_dtypes.bfloat16


@dataclass
class _WTile:
    ki0: tuple | None = None   # (mi, c, w_idx) served by array rows 0-63
    ki1: tuple | None = None   # (mi, c, w_idx) served by array rows 64-127


@dataclass
class _Pair:
    r: tuple                   # (slot0 row, slot1 row)
    wtiles: list = field(default_factory=list)


def _choose_split(M):
    """Pick a 32-col subset L minimizing sum_r |#(cols(r) in L) - 8|."""
    n = IN_BLK
    half = n // 2
    tgt = 8
    best = None
    for seed in range(4):
        rs = np.random.default_rng(seed)
        mask = np.zeros(n, dtype=bool)
        mask[rs.permutation(n)[:half]] = True
        e = M[:, mask].sum(1)
        cost = int(np.abs(e - tgt).sum())
        for _ in range(300):
            Lc = np.nonzero(mask)[0]
            Hc = np.nonzero(~mask)[0]
            ne = e[:, None, None] - M[:, Lc][:, :, None] + M[:, Hc][:, None, :]
            costs = np.abs(ne - tgt).sum(0)
            i, j = np.unravel_index(np.argmin(costs), costs.shape)
            if costs[i, j] >= cost:
                break
            cost = int(costs[i, j])
            e = e - M[:, Lc[i]] + M[:, Hc[j]]
            mask[Lc[i]] = False
            mask[Hc[j]] = True
        if best is None or cost < best[0]:
            best = (cost, mask.copy())
        if best[0] == 0:
            break
    return best[1]


def _make_schedule(row_idx, col_idx):
    # keep-last dedupe of (r, c), matching jax .at[].set semantics
    d = {}
    for i in range(len(row_idx)):
        d[(int(row_idx[i]), int(col_idx[i]))] = i
    blocks_by_r = [[] for _ in range(OUT_BLK)]
    M = np.zeros((OUT_BLK, IN_BLK), dtype=np.int64)
    for (r, c), w in d.items():
        blocks_by_r[r].append((c, w))
        M[r, c] = 1
    for lst in blocks_by_r:
        lst.sort()

    lmask = _choose_split(M)           # side 0 = L (rows 0-63 of PE array)
    side_of = np.where(lmask, 0, 1)
    e = M[:, lmask].sum(1)             # L-count per row

    # pair rows: sort by e, pair extremes -> per-pair L-count ~16
    order = np.argsort(e, kind="stable")
    raw_pairs = [(int(order[i]), int(order[OUT_BLK - 1 - i]))
                 for i in range(OUT_BLK // 2)]

    # slot assignment: keep running sum of slot-0 L-counts near 8/pair
    pairs = []
    q00 = 0
    for idx, (ra, rb) in enumerate(raw_pairs):
        tgt = 8 * (idx + 1)
        if abs(q00 + int(e[ra]) - tgt) <= abs(q00 + int(e[rb]) - tgt):
            s0, s1 = ra, rb
        else:
            s0, s1 = rb, ra
        q00 += int(e[s0])
        ps = _Pair(r=(s0, s1))

        # per-side queues, mi alternating for quadrant diversity
        queues = []
        for side in (0, 1):
            by_mi = [[], []]
            for mi, row in ((0, s0), (1, s1)):
                for (c, w) in blocks_by_r[row]:
                    if side_of[c] == side:
                        by_mi[mi].append((mi, c, w))
            q = []
            t = 0 if len(by_mi[0]) >= len(by_mi[1]) else 1
            while by_mi[0] or by_mi[1]:
                if by_mi[t]:
                    q.append(by_mi[t].pop(0))
                elif by_mi[1 - t]:
                    q.append(by_mi[1 - t].pop(0))
                t = 1 - t
            queues.append(q)

        nw = max(len(queues[0]), len(queues[1]))
        for j in range(nw):
            ps.wtiles.append(_WTile(
                ki0=queues[0][j] if j < len(queues[0]) else None,
                ki1=queues[1][j] if j < len(queues[1]) else None,
            ))
        pairs.append(ps)

    # greedy pair order: next pair = fewest not-yet-touched x cols, so the
    # early x-DMA demand curve stays under the DMA rate
    seen_c = set()
    remaining = list(pairs)
    ordered = []
    while remaining:
        best_i, best_cost = 0, None
        for i, ps in enumerate(remaining):
            cost = len({h[1] for wt in ps.wtiles
                        for h in (wt.ki0, wt.ki1) if h} - seen_c)
            if best_cost is None or cost < best_cost:
                best_i, best_cost = i, cost
        ps = remaining.pop(best_i)
        ordered.append(ps)
        seen_c |= {h[1] for wt in ps.wtiles
                   for h in (wt.ki0, wt.ki1) if h}
    pairs = ordered

    # x tile composition by first touch: tile i = (i-th touched L col,
    # i-th touched H col)
    touched = [[], []]
    seen = set()
    for ps in pairs:
        for wt in ps.wtiles:
            for half in (wt.ki0, wt.ki1):
                if half is None:
                    continue
                c = half[1]
                if c not in seen:
                    seen.add(c)
                    touched[side_of[c]].append(c)
    for c in range(IN_BLK):
        if c not in seen:
            touched[side_of[c]].append(c)
    assert len(touched[0]) == len(touched[1]) == IN_BLK // 2
    tile_of = np.zeros(IN_BLK, dtype=np.int64)
    for i, (cl, ch) in enumerate(zip(touched[0], touched[1])):
        tile_of[cl] = i
        tile_of[ch] = i

    n_wtiles = sum(len(ps.wtiles) for ps in pairs)
    return pairs, n_wtiles, side_of, tile_of, touched


def _pack_host_arrays(weight, bias, pairs, n_wtiles):
    n_groups = (n_wtiles + WGRP - 1) // WGRP
    wgrp = np.zeros((max(n_groups, 1), 128, WGRP * BLK), dtype=BF16)
    bias_pk = np.zeros((128, len(pairs)), dtype=np.float32)
    wT = np.ascontiguousarray(
        np.transpose(np.asarray(weight), (0, 2, 1))).astype(BF16)
    t = 0
    for p, ps in enumerate(pairs):
        r1, r2 = ps.r
        bias_pk[:64, p] = bias[r1 * BLK:(r1 + 1) * BLK]
        bias_pk[64:, p] = bias[r2 * BLK:(r2 + 1) * BLK]
        for wt in ps.wtiles:
            g, j = divmod(t, WGRP)
            for ki, half in ((0, wt.ki0), (1, wt.ki1)):
                if half is not None:
                    wgrp[g, ki * 64:(ki + 1) * 64,
                         j * BLK:(j + 1) * BLK] = wT[half[2]]
            t += 1
    return wgrp, bias_pk


def _build_kernel(pairs, n_wtiles, side_of, tile_of, ntok,
                  w_bufs=None, ps_bufs=8, out_bufs=6):
    assert ntok % 512 == 0
    n_th = ntok // 512
    sdt = mybir.dt.bfloat16
    f32 = mybir.dt.float32

    nc = bacc.Bacc("TRN2", target_bir_lowering=False, debug=False)
    n_groups = (n_wtiles + WGRP - 1) // WGRP
    if w_bufs is None:
        w_bufs = max(n_groups, 1)
    n_chunks = (IN_BLK // 2 + XCH - 1) // XCH
    xt_d = nc.dram_tensor("xt", [D_IN, ntok], sdt,
                          kind="ExternalInput").ap()
    wg_d = nc.dram_tensor("wgrp", [max(n_groups, 1), 128, WGRP * BLK], sdt,
                          kind="ExternalInput").ap()
    bias_d = nc.dram_tensor("bias_pk", [128, len(pairs)], f32,
                            kind="ExternalInput").ap()
    yt_d = nc.dram_tensor("yt", [D_OUT, ntok], sdt,
                          kind="ExternalOutput").ap()

    with tile.TileContext(nc) as tc:
        with ExitStack() as ctx:
            xpool = ctx.enter_context(tc.tile_pool(name="xp", bufs=1))
            wpool = ctx.enter_context(tc.tile_pool(name="wp", bufs=w_bufs))
            pspool = ctx.enter_context(
                tc.tile_pool(name="ps", bufs=ps_bufs, space="PSUM"))
            opool = ctx.enter_context(tc.tile_pool(name="op", bufs=out_bufs))
            bpool = ctx.enter_context(tc.tile_pool(name="bp", bufs=1))

            bias_sb = bpool.tile([128, len(pairs)], f32, tag="bias",
                                 name="bias_sb")
            nc.sync.dma_start(bias_sb[:], bias_d[:])

            # PE warm-up: dependency-free dummy matmuls on scratch SBUF keep
            # the HAM activity monitor busy through the DMA-bound head, so
            # real matmuls start at 2.4GHz instead of 1.2GHz.
            warm_sb = bpool.tile([128, 640], sdt, tag="warm", name="warm_sb")
            nc.vector.memset(warm_sb[:], 0.0)
            warm_ps = pspool.tile([128, 512], f32, tag="ps",
                                  name="warm_ps")
            for wi in range(N_WARM):
                nc.tensor.matmul(
                    warm_ps[:], warm_sb[:, 0:128], warm_sb[:, 128:640],
                    start=True, stop=True, skip_group_check=True)

            # x half-chunks upfront on the sync queue: all th0 halves first
            # (pass 1 streams while pass-2 data still loads)
            xchunks = {}
            for th in range(n_th):
                for cb in range(n_chunks):
                    t = xpool.tile([128, XCH * 512], sdt, tag=f"x{cb}_{th}",
                                   name=f"x{cb}_{th}")
                    src = xt_d[cb * 128 * XCH:(cb + 1) * 128 * XCH,
                               th * 512:(th + 1) * 512].rearrange(
                        "(c p) t -> p c t", p=128)
                    dst = t[:].rearrange("p (c t) -> p c t", c=XCH)
                    nc.sync.dma_start(dst, src)
                    xchunks[(cb, th)] = t

            def x_ap(c, ki, th):
                ti = int(tile_of[c])
                cb, wi = divmod(ti, XCH)
                t = xchunks[(cb, th)]
                o = wi * 512
                return t[ki * 64:(ki + 1) * 64, o:o + 512]

            nmm = [{(ki, mi): 0 for ki in (0, 1) for mi in (0, 1)}
                   for _ in pairs]
            for p, ps_ in enumerate(pairs):
                for wt in ps_.wtiles:
                    for ki, half in ((0, wt.ki0), (1, wt.ki1)):
                        if half is not None:
                            nmm[p][(ki, half[0])] += 1
            done = [{(th, ki, mi): 0 for th in range(n_th)
                     for ki in (0, 1) for mi in (0, 1)} for _ in pairs]

            psum = {}
            wg_tiles = {}

            def ensure_psum(p, th):
                if (p, th) not in psum:
                    psum[(p, th)] = [
                        pspool.tile([128, 512], f32, tag="ps",
                                    name=f"ps{p}_{th}_{k}") for k in range(2)]

            def emit_wtiles(p, ps_, th):
                # one token-half pass over the pair's wtiles: 2 PSUM banks
                # live per pair-phase, so ~4 pair-phases pipeline in 8 banks
                ensure_psum(p, th)
                for wt_j, wt in enumerate(ps_.wtiles):
                    idx = pair_base[p] + wt_j
                    gi = idx // WGRP
                    jj = idx % WGRP
                    for gpf in range(gi, min(gi + W_LOOKAHEAD, n_groups)):
                        if gpf not in wg_tiles:
                            wg_tiles[gpf] = wpool.tile(
                                [128, WGRP * BLK], sdt, tag="wg",
                                name=f"wg{gpf}")
                            nc.scalar.dma_start(wg_tiles[gpf][:],
                                                wg_d[gpf, :, :])
                    for ki, half in ((0, wt.ki0), (1, wt.ki1)):
                        if half is None:
                            continue
                        mi, c, w = half
                        lhsT = wg_tiles[gi][ki * 64:(ki + 1) * 64,
                                            jj * BLK:(jj + 1) * BLK]
                        done[p][(th, ki, mi)] += 1
                        first = done[p][(th, ki, mi)] == 1
                        last = done[p][(th, ki, mi)] == nmm[p][(ki, mi)]
                        nc.tensor.matmul(
                            psum[(p, th)][ki][mi * 64:(mi + 1) * 64, :],
                            lhsT, x_ap(c, ki, th),
                            start=first, stop=last,
                            tile_position=(ki * 64, mi * 64),
                            skip_group_check=True,
                        )

            def eviction_th(p, th):
                osb = opool.tile([128, 512], sdt, tag="o16",
                                 name=f"o{p}_{th}")[:]
                pt = psum.pop((p, th))
                if all(v > 0 for v in nmm[p].values()):
                    nc.scalar.activation(
                        osb, pt[0][:],
                        mybir.ActivationFunctionType.Identity,
                        bias=bias_sb[:, p:p + 1], scale=1.0)
                    nc.vector.tensor_add(osb, osb, pt[1][:])
                else:
                    for mi in (0, 1):
                        oh = osb[mi * 64:(mi + 1) * 64, :]
                        bh = bias_sb[mi * 64:(mi + 1) * 64, p:p + 1]
                        srcs = [pt[ki][mi * 64:(mi + 1) * 64, :]
                                for ki in (0, 1) if nmm[p][(ki, mi)] > 0]
                        if not srcs:
                            nc.vector.memset(oh, 0.0)
                            nc.vector.tensor_scalar_add(oh, oh, bh)
                        else:
                            nc.scalar.activation(
                                oh, srcs[0],
                                mybir.ActivationFunctionType.Identity,
                                bias=bh, scale=1.0)
                            if len(srcs) > 1:
                                nc.vector.tensor_add(oh, oh, srcs[1])
                nc.sync.dma_start(
                    yt_d[p * 128:(p + 1) * 128, th * 512:(th + 1) * 512],
                    osb)

            pair_base = []
            acc = 0
            for ps_ in pairs:
                pair_base.append(acc)
                acc += len(ps_.wtiles)

            for th in range(n_th):
                for p, ps_ in enumerate(pairs):
                    if not ps_.wtiles:
                        continue
                    emit_wtiles(p, ps_, th)
                    eviction_th(p, th)

            for p, ps_ in enumerate(pairs):
                if ps_.wtiles:
                    continue
                for th in range(n_th):
                    osb = opool.tile([128, 512], sdt, tag="o16",
                                     name=f"oz{p}_{th}")
                    nc.vector.memset(osb[:], 0.0)
                    nc.vector.tensor_scalar_add(osb[:], osb[:],
                                                bias_sb[:, p:p + 1])
                    nc.sync.dma_start(
                        yt_d[p * 128:(p + 1) * 128,
                             th * 512:(th + 1) * 512], osb[:])
    nc.compile()
    return nc


def kernel(x, weight, bias, row_idx, col_idx):
    x = np.asarray(x, dtype=np.float32)
    weight = np.asarray(weight, dtype=np.float32)
    bias = np.asarray(bias, dtype=np.float32)
    row_idx = np.asarray(row_idx)
    col_idx = np.asarray(col_idx)
    ntok_total = x.shape[0]
    assert ntok_total % N_CORES == 0
    ntok = ntok_total // N_CORES

    pairs, n_wt, side_of, tile_of, touched = _make_schedule(row_idx, col_idx)
    wgrp, bias_pk = _pack_host_arrays(weight, bias, pairs, n_wt)
    nc = _build_kernel(pairs, n_wt, side_of, tile_of, ntok)

    # tile-major x^T: tile i rows = [block touched[0][i]; block touched[1][i]]
    perm = np.empty(IN_BLK, dtype=np.int64)
    for i in range(IN_BLK // 2):
        perm[2 * i] = touched[0][i]
        perm[2 * i + 1] = touched[1][i]
    xT = np.ascontiguousarray(x.T).astype(BF16)      # [D_IN, ntok_total]
    xt_full = xT.reshape(IN_BLK, BLK, ntok_total)[perm].reshape(
        D_IN, ntok_total)

    in_maps = []
    for c in range(N_CORES):
        xt = np.ascontiguousarray(xt_full[:, c * ntok:(c + 1) * ntok])
        in_maps.append({"xt": xt, "wgrp": wgrp, "bias_pk": bias_pk})

    res = run_bass_kernel_spmd(nc, in_maps, core_ids=list(range(N_CORES)))

    # un-permute pair-major rows, transpose, upcast
    row_perm = np.empty(D_OUT, dtype=np.int64)
    for p, ps in enumerate(pairs):
        r1, r2 = ps.r
        row_perm[r1 * BLK:(r1 + 1) * BLK] = np.arange(
            p * 128, p * 128 + 64)
        row_perm[r2 * BLK:(r2 + 1) * BLK] = np.arange(
            p * 128 + 64, p * 128 + 128)
    y = np.empty((ntok_total, D_OUT), dtype=np.float32)
    for c in range(N_CORES):
        yt = np.asarray(res.results[c]["yt"])          # [D_OUT, ntok] bf16
        y[c * ntok:(c + 1) * ntok] = yt[row_perm].T.astype(np.float32)
    return y


# revision 16
# speedup vs baseline: 1.1665x; 1.1665x over previous
"""Block-sparse linear y = x @ W^T + b on 8 TRN2 NeuronCores.

Problem shape (hardcoded): x [8192, 4096] f32, weight [1024, 64, 64] f32
(64x64 blocks), bias [4096] f32, row_idx/col_idx [1024] int32 over a 64x64
block grid.

Strategy: data-parallel over tokens (1024/core). Each core computes
y^T[feat, tok] = W x^T + b via 64x64 block matmuls packed 4-wide into the
PE-array quadrants with tile_position; compute in bf16 (fp32 PSUM
accumulate).

v2 over the baseline:
- A 32/32 column-block split L/H (chosen by local search so every row has
  ~8 blocks per side) fixes each block's array row-group: side L -> rows
  0-63, side H -> rows 64-127. x^T is packed tile-major on host (tile i =
  [x_{l_i}; x_{h_i}]), so x is DMA'd ONCE (8.4MB instead of the baseline's
  16.8MB double copy) and all x DMAs are issued upfront in first-use order.
- Row pairs are matched so each pair has ~16 blocks per side, and slot
  assignment balances the 4 PE quadrants globally.
- Weight groups stream on the scalar engine's DMA queue (x owns the sync
  queue) with 4-group lookahead.
- Output is stored bf16 in pair-major rows (one [128, 512] DMA per
  eviction); eviction sums the two PSUM banks on VectorE, then ScalarE
  applies bias and casts to bf16. Host un-permutes/transposes/upcasts.

TRN2 PSUM rule (measured): concurrent matmuls that share an output
col-group must write different PSUM banks -> row-group ki writes psum
tile[ki].
"""

from contextlib import ExitStack
from dataclasses import dataclass, field

import numpy as np
import ml_dtypes

import concourse.tile as tile
from concourse import bacc, mybir
from concourse.bass_utils import run_bass_kernel_spmd

BLK = 64
OUT_BLK = 64
IN_BLK = 64
D_IN = IN_BLK * BLK    # 4096
D_OUT = OUT_BLK * BLK  # 4096
N_CORES = 8
WGRP = 16              # weight tiles per DMA group
XCH = 2                # x tiles (128-row groups) per DMA chunk
W_LOOKAHEAD = 4        # weight groups prefetched ahead
N_WARM = 8             # dummy matmuls to warm the HAM clock before real MMs
BF16 = ml_dtypes.bfloat16


@dataclass
class _WTile:
    ki0: tuple | None = None   # (mi, c, w_idx) served by array rows 0-63
    ki1: tuple | None = None   # (mi, c, w_idx) served by array rows 64-127


@dataclass
class _Pair:
    r: tuple                   # (slot0 row, slot1 row)
    wtiles: list = field(default_factory=list)


def _choose_split(M):
    """Pick a 32-col subset L minimizing sum_r |#(cols(r) in L) - 8|."""
    n = IN_BLK
    half = n // 2
    tgt = 8
    best = None
    for seed in range(4):
        rs = np.random.default_rng(seed)
        mask = np.zeros(n, dtype=bool)
        mask[rs.permutation(n)[:half]] = True
        e = M[:, mask].sum(1)
        cost = int(np.abs(e - tgt).sum())
        for _ in range(300):
            Lc = np.nonzero(mask)[0]
            Hc = np.nonzero(~mask)[0]
            ne = e[:, None, None] - M[:, Lc][:, :, None] + M[:, Hc][:, None, :]
            costs = np.abs(ne - tgt).sum(0)
            i, j = np.unravel_index(np.argmin(costs), costs.shape)
            if costs[i, j] >= cost:
                break
            cost = int(costs[i, j])
            e = e - M[:, Lc[i]] + M[:, Hc[j]]
            mask[Lc[i]] = False
            mask[Hc[j]] = True
        if best is None or cost < best[0]:
            best = (cost, mask.copy())
        if best[0] == 0:
            break
    return best[1]


def _make_schedule(row_idx, col_idx):
    # keep-last dedupe of (r, c), matching jax .at[].set semantics
    d = {}
    for i in range(len(row_idx)):
        d[(int(row_idx[i]), int(col_idx[i]))] = i
    blocks_by_r = [[] for _ in range(OUT_BLK)]
    M = np.zeros((OUT_BLK, IN_BLK), dtype=np.int64)
    for (r, c), w in d.items():
        blocks_by_r[r].append((c, w))
        M[r, c] = 1
    for lst in blocks_by_r:
        lst.sort()

    lmask = _choose_split(M)           # side 0 = L (rows 0-63 of PE array)
    side_of = np.where(lmask, 0, 1)
    e = M[:, lmask].sum(1)             # L-count per row

    # pair rows: sort by e, pair extremes -> per-pair L-count ~16
    order = np.argsort(e, kind="stable")
    raw_pairs = [(int(order[i]), int(order[OUT_BLK - 1 - i]))
                 for i in range(OUT_BLK // 2)]

    # slot assignment: keep running sum of slot-0 L-counts near 8/pair
    pairs = []
    q00 = 0
    for idx, (ra, rb) in enumerate(raw_pairs):
        tgt = 8 * (idx + 1)
        if abs(q00 + int(e[ra]) - tgt) <= abs(q00 + int(e[rb]) - tgt):
            s0, s1 = ra, rb
        else:
            s0, s1 = rb, ra
        q00 += int(e[s0])
        ps = _Pair(r=(s0, s1))

        # per-side queues, mi alternating for quadrant diversity
        queues = []
        for side in (0, 1):
            by_mi = [[], []]
            for mi, row in ((0, s0), (1, s1)):
                for (c, w) in blocks_by_r[row]:
                    if side_of[c] == side:
                        by_mi[mi].append((mi, c, w))
            q = []
            t = 0 if len(by_mi[0]) >= len(by_mi[1]) else 1
            while by_mi[0] or by_mi[1]:
                if by_mi[t]:
                    q.append(by_mi[t].pop(0))
                elif by_mi[1 - t]:
                    q.append(by_mi[1 - t].pop(0))
                t = 1 - t
            queues.append(q)

        nw = max(len(queues[0]), len(queues[1]))
        for j in range(nw):
            ps.wtiles.append(_WTile(
                ki0=queues[0][j] if j < len(queues[0]) else None,
                ki1=queues[1][j] if j < len(queues[1]) else None,
            ))
        pairs.append(ps)

    # greedy pair order: next pair = fewest not-yet-touched x cols, so the
    # early x-DMA demand curve stays under the DMA rate
    seen_c = set()
    remaining = list(pairs)
    ordered = []
    while remaining:
        best_i, best_cost = 0, None
        for i, ps in enumerate(remaining):
            cost = len({h[1] for wt in ps.wtiles
                        for h in (wt.ki0, wt.ki1) if h} - seen_c)
            if best_cost is None or cost < best_cost:
                best_i, best_cost = i, cost
        ps = remaining.pop(best_i)
        ordered.append(ps)
        seen_c |= {h[1] for wt in ps.wtiles
                   for h in (wt.ki0, wt.ki1) if h}
    pairs = ordered

    # x tile composition by first touch: tile i = (i-th touched L col,
    # i-th touched H col)
    touched = [[], []]
    seen = set()
    for ps in pairs:
        for wt in ps.wtiles:
            for half in (wt.ki0, wt.ki1):
                if half is None:
                    continue
                c = half[1]
                if c not in seen:
                    seen.add(c)
                    touched[side_of[c]].append(c)
    for c in range(IN_BLK):
        if c not in seen:
            touched[side_of[c]].append(c)
    assert len(touched[0]) == len(touched[1]) == IN_BLK // 2
    tile_of = np.zeros(IN_BLK, dtype=np.int64)
    for i, (cl, ch) in enumerate(zip(touched[0], touched[1])):
        tile_of[cl] = i
        tile_of[ch] = i

    n_wtiles = sum(len(ps.wtiles) for ps in pairs)
    return pairs, n_wtiles, side_of, tile_of, touched


def _pack_host_arrays(weight, bias, pairs, n_wtiles):
    n_groups = (n_wtiles + WGRP - 1) // WGRP
    wgrp = np.zeros((max(n_groups, 1), 128, WGRP * BLK), dtype=BF16)
    bias_pk = np.zeros((128, len(pairs)), dtype=np.float32)
    wT = np.ascontiguousarray(
        np.transpose(np.asarray(weight), (0, 2, 1))).astype(BF16)
    t = 0
    for p, ps in enumerate(pairs):
        r1, r2 = ps.r
        bias_pk[:64, p] = bias[r1 * BLK:(r1 + 1) * BLK]
        bias_pk[64:, p] = bias[r2 * BLK:(r2 + 1) * BLK]
        for wt in ps.wtiles:
            g, j = divmod(t, WGRP)
            for ki, half in ((0, wt.ki0), (1, wt.ki1)):
                if half is not None:
                    wgrp[g, ki * 64:(ki + 1) * 64,
                         j * BLK:(j + 1) * BLK] = wT[half[2]]
            t += 1
    return wgrp, bias_pk


def _build_kernel(pairs, n_wtiles, side_of, tile_of, ntok,
                  w_bufs=6, ps_bufs=8, out_bufs=6):
    assert ntok % 512 == 0
    n_th = ntok // 512
    sdt = mybir.dt.bfloat16
    f32 = mybir.dt.float32

    nc = bacc.Bacc("TRN2", target_bir_lowering=False, debug=False)
    n_groups = (n_wtiles + WGRP - 1) // WGRP
    n_chunks = (IN_BLK // 2 + XCH - 1) // XCH
    xt_d = nc.dram_tensor("xt", [D_IN, ntok], sdt,
                          kind="ExternalInput").ap()
    wg_d = nc.dram_tensor("wgrp", [max(n_groups, 1), 128, WGRP * BLK], sdt,
                          kind="ExternalInput").ap()
    bias_d = nc.dram_tensor("bias_pk", [128, len(pairs)], f32,
                            kind="ExternalInput").ap()
    yt_d = nc.dram_tensor("yt", [D_OUT, ntok], sdt,
                          kind="ExternalOutput").ap()

    with tile.TileContext(nc) as tc:
        with ExitStack() as ctx:
            xpool = ctx.enter_context(tc.tile_pool(name="xp", bufs=1))
            wpool = ctx.enter_context(tc.tile_pool(name="wp", bufs=w_bufs))
            pspool = ctx.enter_context(
                tc.tile_pool(name="ps", bufs=ps_bufs, space="PSUM"))
            opool = ctx.enter_context(tc.tile_pool(name="op", bufs=out_bufs))
            bpool = ctx.enter_context(tc.tile_pool(name="bp", bufs=1))

            bias_sb = bpool.tile([128, len(pairs)], f32, tag="bias",
                                 name="bias_sb")
            nc.sync.dma_start(bias_sb[:], bias_d[:])

            # PE warm-up: dependency-free dummy matmuls on scratch SBUF keep
            # the HAM activity monitor busy through the DMA-bound head, so
            # real matmuls start at 2.4GHz instead of 1.2GHz.
            warm_sb = bpool.tile([128, 640], sdt, tag="warm", name="warm_sb")
            nc.vector.memset(warm_sb[:], 0.0)
            warm_ps = pspool.tile([128, 512], f32, tag="ps",
                                  name="warm_ps")
            for wi in range(N_WARM):
                nc.tensor.matmul(
                    warm_ps[:], warm_sb[:, 0:128], warm_sb[:, 128:640],
                    start=True, stop=True, skip_group_check=True)

            # all x chunks upfront, in consumption (tile) order, sync queue
            xchunks = []
            for cb in range(n_chunks):
                t = xpool.tile([128, XCH * ntok], sdt, tag=f"x{cb}",
                               name=f"x{cb}")
                src = xt_d[cb * 128 * XCH:(cb + 1) * 128 * XCH, :].rearrange(
                    "(c p) t -> p c t", p=128)
                dst = t[:].rearrange("p (c t) -> p c t", c=XCH)
                nc.sync.dma_start(dst, src)
                xchunks.append(t)

            def x_ap(c, ki, th):
                ti = int(tile_of[c])
                cb, wi = divmod(ti, XCH)
                t = xchunks[cb]
                o = wi * ntok + th * 512
                return t[ki * 64:(ki + 1) * 64, o:o + 512]

            nmm = [{(ki, mi): 0 for ki in (0, 1) for mi in (0, 1)}
                   for _ in pairs]
            for p, ps_ in enumerate(pairs):
                for wt in ps_.wtiles:
                    for ki, half in ((0, wt.ki0), (1, wt.ki1)):
                        if half is not None:
                            nmm[p][(ki, half[0])] += 1
            done = [{(th, ki, mi): 0 for th in range(n_th)
                     for ki in (0, 1) for mi in (0, 1)} for _ in pairs]

            psum = {}
            wg_tiles = {}

            def ensure_psum(p, th):
                if (p, th) not in psum:
                    psum[(p, th)] = [
                        pspool.tile([128, 512], f32, tag="ps",
                                    name=f"ps{p}_{th}_{k}") for k in range(2)]

            def emit_pair(p, ps_):
                for th in range(n_th):
                    ensure_psum(p, th)
                for wt_j, wt in enumerate(ps_.wtiles):
                    idx = pair_base[p] + wt_j
                    gi = idx // WGRP
                    jj = idx % WGRP
                    for gpf in range(gi, min(gi + W_LOOKAHEAD, n_groups)):
                        if gpf not in wg_tiles:
                            wg_tiles[gpf] = wpool.tile(
                                [128, WGRP * BLK], sdt, tag="wg",
                                name=f"wg{gpf}")
                            nc.scalar.dma_start(wg_tiles[gpf][:],
                                                wg_d[gpf, :, :])
                    for ki, half in ((0, wt.ki0), (1, wt.ki1)):
                        if half is None:
                            continue
                        mi, c, w = half
                        lhsT = wg_tiles[gi][ki * 64:(ki + 1) * 64,
                                            jj * BLK:(jj + 1) * BLK]
                        for th in range(n_th):
                            done[p][(th, ki, mi)] += 1
                            first = done[p][(th, ki, mi)] == 1
                            last = done[p][(th, ki, mi)] == nmm[p][(ki, mi)]
                            nc.tensor.matmul(
                                psum[(p, th)][ki][mi * 64:(mi + 1) * 64, :],
                                lhsT, x_ap(c, ki, th),
                                start=first, stop=last,
                                tile_position=(ki * 64, mi * 64),
                                skip_group_check=True,
                            )

            osb_cur = {}

            def eviction_th(p, th):
                if p not in osb_cur:
                    osb_cur[p] = opool.tile([128, n_th * 512], sdt,
                                            tag="o16", name=f"o{p}")
                osb = osb_cur[p][:, th * 512:(th + 1) * 512]
                pt = psum.pop((p, th))
                if all(v > 0 for v in nmm[p].values()):
                    nc.scalar.activation(
                        osb, pt[0][:],
                        mybir.ActivationFunctionType.Identity,
                        bias=bias_sb[:, p:p + 1], scale=1.0)
                    nc.vector.tensor_add(osb, osb, pt[1][:])
                else:
                    for mi in (0, 1):
                        oh = osb[mi * 64:(mi + 1) * 64, :]
                        bh = bias_sb[mi * 64:(mi + 1) * 64, p:p + 1]
                        srcs = [pt[ki][mi * 64:(mi + 1) * 64, :]
                                for ki in (0, 1) if nmm[p][(ki, mi)] > 0]
                        if not srcs:
                            nc.vector.memset(oh, 0.0)
                            nc.vector.tensor_scalar_add(oh, oh, bh)
                        else:
                            nc.scalar.activation(
                                oh, srcs[0],
                                mybir.ActivationFunctionType.Identity,
                                bias=bh, scale=1.0)
                            if len(srcs) > 1:
                                nc.vector.tensor_add(oh, oh, srcs[1])
                if th == n_th - 1:
                    nc.sync.dma_start(yt_d[p * 128:(p + 1) * 128, :],
                                      osb_cur.pop(p)[:])

            pair_base = []
            acc = 0
            for ps_ in pairs:
                pair_base.append(acc)
                acc += len(ps_.wtiles)

            for p, ps_ in enumerate(pairs):
                if not ps_.wtiles:
                    continue
                emit_pair(p, ps_)
                for th in range(n_th):
                    eviction_th(p, th)

            for p, ps_ in enumerate(pairs):
                if ps_.wtiles:
                    continue
                osb = opool.tile([128, n_th * 512], sdt, tag="o16",
                                 name=f"oz{p}")
                nc.vector.memset(osb[:], 0.0)
                nc.vector.tensor_scalar_add(osb[:], osb[:],
                                            bias_sb[:, p:p + 1])
                nc.sync.dma_start(yt_d[p * 128:(p + 1) * 128, :], osb[:])
    nc.compile()
    return nc


def kernel(x, weight, bias, row_idx, col_idx):
    x = np.asarray(x, dtype=np.float32)
    weight = np.asarray(weight, dtype=np.float32)
    bias = np.asarray(bias, dtype=np.float32)
    row_idx = np.asarray(row_idx)
    col_idx = np.asarray(col_idx)
    ntok_total = x.shape[0]
    assert ntok_total % N_CORES == 0
    ntok = ntok_total // N_CORES

    pairs, n_wt, side_of, tile_of, touched = _make_schedule(row_idx, col_idx)
    wgrp, bias_pk = _pack_host_arrays(weight, bias, pairs, n_wt)
    nc = _build_kernel(pairs, n_wt, side_of, tile_of, ntok)

    # tile-major x^T: tile i rows = [block touched[0][i]; block touched[1][i]]
    perm = np.empty(IN_BLK, dtype=np.int64)
    for i in range(IN_BLK // 2):
        perm[2 * i] = touched[0][i]
        perm[2 * i + 1] = touched[1][i]
    xT = np.ascontiguousarray(x.T).astype(BF16)      # [D_IN, ntok_total]
    xt_full = xT.reshape(IN_BLK, BLK, ntok_total)[perm].reshape(
        D_IN, ntok_total)

    in_maps = []
    for c in range(N_CORES):
        xt = np.ascontiguousarray(xt_full[:, c * ntok:(c + 1) * ntok])
        in_maps.append({"xt": xt, "wgrp": wgrp, "bias_pk": bias_pk})

    res = run_bass_kernel_spmd(nc, in_maps, core_ids=list(range(N_CORES)))

    # un-permute pair-major rows, transpose, upcast
    row_perm = np.empty(D_OUT, dtype=np.int64)
    for p, ps in enumerate(pairs):
        r1, r2 = ps.r
        row_perm[r1 * BLK:(r1 + 1) * BLK] = np.arange(
            p * 128, p * 128 + 64)
        row_perm[r2 * BLK:(r2 + 1) * BLK] = np.arange(
            p * 128 + 64, p * 128 + 128)
    y = np.empty((ntok_total, D_OUT), dtype=np.float32)
    for c in range(N_CORES):
        yt = np.asarray(res.results[c]["yt"])          # [D_OUT, ntok] bf16
        y[c * ntok:(c + 1) * ntok] = yt[row_perm].T.astype(np.float32)
    return y


# revision 17
# speedup vs baseline: 1.1789x; 1.0107x over previous
"""Block-sparse linear y = x @ W^T + b on 8 TRN2 NeuronCores.

Problem shape (hardcoded): x [8192, 4096] f32, weight [1024, 64, 64] f32
(64x64 blocks), bias [4096] f32, row_idx/col_idx [1024] int32 over a 64x64
block grid.

Strategy: data-parallel over tokens (1024/core). Each core computes
y^T[feat, tok] = W x^T + b via 64x64 block matmuls packed 4-wide into the
PE-array quadrants with tile_position; compute in bf16 (fp32 PSUM
accumulate).

v2 over the baseline:
- A 32/32 column-block split L/H (chosen by local search so every row has
  ~8 blocks per side) fixes each block's array row-group: side L -> rows
  0-63, side H -> rows 64-127. x^T is packed tile-major on host (tile i =
  [x_{l_i}; x_{h_i}]), so x is DMA'd ONCE (8.4MB instead of the baseline's
  16.8MB double copy) and all x DMAs are issued upfront in first-use order.
- Row pairs are matched so each pair has ~16 blocks per side, and slot
  assignment balances the 4 PE quadrants globally.
- Weight groups stream on the scalar engine's DMA queue (x owns the sync
  queue) with 4-group lookahead.
- Output is stored bf16 in pair-major rows (one [128, 512] DMA per
  eviction); eviction sums the two PSUM banks on VectorE, then ScalarE
  applies bias and casts to bf16. Host un-permutes/transposes/upcasts.

TRN2 PSUM rule (measured): concurrent matmuls that share an output
col-group must write different PSUM banks -> row-group ki writes psum
tile[ki].
"""

from contextlib import ExitStack
from dataclasses import dataclass, field

import numpy as np
import ml_dtypes

import concourse.tile as tile
from concourse import bacc, mybir
from concourse.bass_utils import run_bass_kernel_spmd

BLK = 64
OUT_BLK = 64
IN_BLK = 64
D_IN = IN_BLK * BLK    # 4096
D_OUT = OUT_BLK * BLK  # 4096
N_CORES = 8
WGRP = 16              # weight tiles per DMA group
XCH = 2                # x tiles (128-row groups) per DMA chunk
W_LOOKAHEAD = 4        # weight groups prefetched ahead
N_WARM = 8             # dummy matmuls to warm the HAM clock before real MMs
BF16 = ml_dtypes.bfloat16


@dataclass
class _WTile:
    ki0: tuple | None = None   # (mi, c, w_idx) served by array rows 0-63
    ki1: tuple | None = None   # (mi, c, w_idx) served by array rows 64-127


@dataclass
class _Pair:
    r: tuple                   # (slot0 row, slot1 row)
    wtiles: list = field(default_factory=list)


def _choose_split(M):
    """Pick a 32-col subset L minimizing sum_r |#(cols(r) in L) - 8|."""
    n = IN_BLK
    half = n // 2
    tgt = 8
    best = None
    for seed in range(4):
        rs = np.random.default_rng(seed)
        mask = np.zeros(n, dtype=bool)
        mask[rs.permutation(n)[:half]] = True
        e = M[:, mask].sum(1)
        cost = int(np.abs(e - tgt).sum())
        for _ in range(300):
            Lc = np.nonzero(mask)[0]
            Hc = np.nonzero(~mask)[0]
            ne = e[:, None, None] - M[:, Lc][:, :, None] + M[:, Hc][:, None, :]
            costs = np.abs(ne - tgt).sum(0)
            i, j = np.unravel_index(np.argmin(costs), costs.shape)
            if costs[i, j] >= cost:
                break
            cost = int(costs[i, j])
            e = e - M[:, Lc[i]] + M[:, Hc[j]]
            mask[Lc[i]] = False
            mask[Hc[j]] = True
        if best is None or cost < best[0]:
            best = (cost, mask.copy())
        if best[0] == 0:
            break
    return best[1]


def _make_schedule(row_idx, col_idx):
    # keep-last dedupe of (r, c), matching jax .at[].set semantics
    d = {}
    for i in range(len(row_idx)):
        d[(int(row_idx[i]), int(col_idx[i]))] = i
    blocks_by_r = [[] for _ in range(OUT_BLK)]
    M = np.zeros((OUT_BLK, IN_BLK), dtype=np.int64)
    for (r, c), w in d.items():
        blocks_by_r[r].append((c, w))
        M[r, c] = 1
    for lst in blocks_by_r:
        lst.sort()

    lmask = _choose_split(M)           # side 0 = L (rows 0-63 of PE array)
    side_of = np.where(lmask, 0, 1)
    e = M[:, lmask].sum(1)             # L-count per row

    # pair rows: sort by e, pair extremes -> per-pair L-count ~16
    order = np.argsort(e, kind="stable")
    raw_pairs = [(int(order[i]), int(order[OUT_BLK - 1 - i]))
                 for i in range(OUT_BLK // 2)]

    # slot assignment: keep running sum of slot-0 L-counts near 8/pair
    pairs = []
    q00 = 0
    for idx, (ra, rb) in enumerate(raw_pairs):
        tgt = 8 * (idx + 1)
        if abs(q00 + int(e[ra]) - tgt) <= abs(q00 + int(e[rb]) - tgt):
            s0, s1 = ra, rb
        else:
            s0, s1 = rb, ra
        q00 += int(e[s0])
        ps = _Pair(r=(s0, s1))

        # per-side queues, mi alternating for quadrant diversity
        queues = []
        for side in (0, 1):
            by_mi = [[], []]
            for mi, row in ((0, s0), (1, s1)):
                for (c, w) in blocks_by_r[row]:
                    if side_of[c] == side:
                        by_mi[mi].append((mi, c, w))
            q = []
            t = 0 if len(by_mi[0]) >= len(by_mi[1]) else 1
            while by_mi[0] or by_mi[1]:
                if by_mi[t]:
                    q.append(by_mi[t].pop(0))
                elif by_mi[1 - t]:
                    q.append(by_mi[1 - t].pop(0))
                t = 1 - t
            queues.append(q)

        nw = max(len(queues[0]), len(queues[1]))
        for j in range(nw):
            ps.wtiles.append(_WTile(
                ki0=queues[0][j] if j < len(queues[0]) else None,
                ki1=queues[1][j] if j < len(queues[1]) else None,
            ))
        pairs.append(ps)

    # greedy pair order: next pair = fewest not-yet-touched x cols, so the
    # early x-DMA demand curve stays under the DMA rate
    seen_c = set()
    remaining = list(pairs)
    ordered = []
    while remaining:
        best_i, best_cost = 0, None
        for i, ps in enumerate(remaining):
            cost = len({h[1] for wt in ps.wtiles
                        for h in (wt.ki0, wt.ki1) if h} - seen_c)
            if best_cost is None or cost < best_cost:
                best_i, best_cost = i, cost
        ps = remaining.pop(best_i)
        ordered.append(ps)
        seen_c |= {h[1] for wt in ps.wtiles
                   for h in (wt.ki0, wt.ki1) if h}
    pairs = ordered

    # x tile composition by first touch: tile i = (i-th touched L col,
    # i-th touched H col)
    touched = [[], []]
    seen = set()
    for ps in pairs:
        for wt in ps.wtiles:
            for half in (wt.ki0, wt.ki1):
                if half is None:
                    continue
                c = half[1]
                if c not in seen:
                    seen.add(c)
                    touched[side_of[c]].append(c)
    for c in range(IN_BLK):
        if c not in seen:
            touched[side_of[c]].append(c)
    assert len(touched[0]) == len(touched[1]) == IN_BLK // 2
    tile_of = np.zeros(IN_BLK, dtype=np.int64)
    for i, (cl, ch) in enumerate(zip(touched[0], touched[1])):
        tile_of[cl] = i
        tile_of[ch] = i

    # rebuild each pair's queues in tile-arrival order (mi still alternating)
    # so every pair consumes x chunks in the order they land
    for ps in pairs:
        queues = []
        for side_halves in ((wt.ki0 for wt in ps.wtiles),
                            (wt.ki1 for wt in ps.wtiles)):
            by_mi = [[], []]
            for h in side_halves:
                if h is not None:
                    by_mi[h[0]].append(h)
            for lst in by_mi:
                lst.sort(key=lambda h: tile_of[h[1]])
            q = []
            t = 0 if len(by_mi[0]) >= len(by_mi[1]) else 1
            while by_mi[0] or by_mi[1]:
                if by_mi[t]:
                    q.append(by_mi[t].pop(0))
                elif by_mi[1 - t]:
                    q.append(by_mi[1 - t].pop(0))
                t = 1 - t
            queues.append(q)
        nw = max(len(queues[0]), len(queues[1]))
        ps.wtiles = [
            _WTile(ki0=queues[0][j] if j < len(queues[0]) else None,
                   ki1=queues[1][j] if j < len(queues[1]) else None)
            for j in range(nw)]

    n_wtiles = sum(len(ps.wtiles) for ps in pairs)
    return pairs, n_wtiles, side_of, tile_of, touched


def _pack_host_arrays(weight, bias, pairs, n_wtiles):
    n_groups = (n_wtiles + WGRP - 1) // WGRP
    wgrp = np.zeros((max(n_groups, 1), 128, WGRP * BLK), dtype=BF16)
    bias_pk = np.zeros((128, len(pairs)), dtype=np.float32)
    wT = np.ascontiguousarray(
        np.transpose(np.asarray(weight), (0, 2, 1))).astype(BF16)
    t = 0
    for p, ps in enumerate(pairs):
        r1, r2 = ps.r
        bias_pk[:64, p] = bias[r1 * BLK:(r1 + 1) * BLK]
        bias_pk[64:, p] = bias[r2 * BLK:(r2 + 1) * BLK]
        for wt in ps.wtiles:
            g, j = divmod(t, WGRP)
            for ki, half in ((0, wt.ki0), (1, wt.ki1)):
                if half is not None:
                    wgrp[g, ki * 64:(ki + 1) * 64,
                         j * BLK:(j + 1) * BLK] = wT[half[2]]
            t += 1
    return wgrp, bias_pk


def _build_kernel(pairs, n_wtiles, side_of, tile_of, ntok,
                  w_bufs=6, ps_bufs=8, out_bufs=6):
    assert ntok % 512 == 0
    n_th = ntok // 512
    sdt = mybir.dt.bfloat16
    f32 = mybir.dt.float32

    nc = bacc.Bacc("TRN2", target_bir_lowering=False, debug=False)
    n_groups = (n_wtiles + WGRP - 1) // WGRP
    n_chunks = (IN_BLK // 2 + XCH - 1) // XCH
    xt_d = nc.dram_tensor("xt", [D_IN, ntok], sdt,
                          kind="ExternalInput").ap()
    wg_d = nc.dram_tensor("wgrp", [max(n_groups, 1), 128, WGRP * BLK], sdt,
                          kind="ExternalInput").ap()
    bias_d = nc.dram_tensor("bias_pk", [128, len(pairs)], f32,
                            kind="ExternalInput").ap()
    yt_d = nc.dram_tensor("yt", [D_OUT, ntok], sdt,
                          kind="ExternalOutput").ap()

    with tile.TileContext(nc) as tc:
        with ExitStack() as ctx:
            xpool = ctx.enter_context(tc.tile_pool(name="xp", bufs=1))
            wpool = ctx.enter_context(tc.tile_pool(name="wp", bufs=w_bufs))
            pspool = ctx.enter_context(
                tc.tile_pool(name="ps", bufs=ps_bufs, space="PSUM"))
            opool = ctx.enter_context(tc.tile_pool(name="op", bufs=out_bufs))
            bpool = ctx.enter_context(tc.tile_pool(name="bp", bufs=1))

            bias_sb = bpool.tile([128, len(pairs)], f32, tag="bias",
                                 name="bias_sb")
            nc.sync.dma_start(bias_sb[:], bias_d[:])

            # PE warm-up: dependency-free dummy matmuls on scratch SBUF keep
            # the HAM activity monitor busy through the DMA-bound head, so
            # real matmuls start at 2.4GHz instead of 1.2GHz.
            warm_sb = bpool.tile([128, 640], sdt, tag="warm", name="warm_sb")
            nc.vector.memset(warm_sb[:], 0.0)
            warm_ps = pspool.tile([128, 512], f32, tag="ps",
                                  name="warm_ps")
            for wi in range(N_WARM):
                nc.tensor.matmul(
                    warm_ps[:], warm_sb[:, 0:128], warm_sb[:, 128:640],
                    start=True, stop=True, skip_group_check=True)

            # all x chunks upfront, in consumption (tile) order, sync queue
            xchunks = []
            for cb in range(n_chunks):
                t = xpool.tile([128, XCH * ntok], sdt, tag=f"x{cb}",
                               name=f"x{cb}")
                src = xt_d[cb * 128 * XCH:(cb + 1) * 128 * XCH, :].rearrange(
                    "(c p) t -> p c t", p=128)
                dst = t[:].rearrange("p (c t) -> p c t", c=XCH)
                nc.sync.dma_start(dst, src)
                xchunks.append(t)

            def x_ap(c, ki, th):
                ti = int(tile_of[c])
                cb, wi = divmod(ti, XCH)
                t = xchunks[cb]
                o = wi * ntok + th * 512
                return t[ki * 64:(ki + 1) * 64, o:o + 512]

            nmm = [{(ki, mi): 0 for ki in (0, 1) for mi in (0, 1)}
                   for _ in pairs]
            for p, ps_ in enumerate(pairs):
                for wt in ps_.wtiles:
                    for ki, half in ((0, wt.ki0), (1, wt.ki1)):
                        if half is not None:
                            nmm[p][(ki, half[0])] += 1
            done = [{(th, ki, mi): 0 for th in range(n_th)
                     for ki in (0, 1) for mi in (0, 1)} for _ in pairs]

            psum = {}
            wg_tiles = {}

            def ensure_psum(p, th):
                if (p, th) not in psum:
                    psum[(p, th)] = [
                        pspool.tile([128, 512], f32, tag="ps",
                                    name=f"ps{p}_{th}_{k}") for k in range(2)]

            def emit_pair(p, ps_):
                for th in range(n_th):
                    ensure_psum(p, th)
                for wt_j, wt in enumerate(ps_.wtiles):
                    idx = pair_base[p] + wt_j
                    gi = idx // WGRP
                    jj = idx % WGRP
                    for gpf in range(gi, min(gi + W_LOOKAHEAD, n_groups)):
                        if gpf not in wg_tiles:
                            wg_tiles[gpf] = wpool.tile(
                                [128, WGRP * BLK], sdt, tag="wg",
                                name=f"wg{gpf}")
                            nc.scalar.dma_start(wg_tiles[gpf][:],
                                                wg_d[gpf, :, :])
                    for ki, half in ((0, wt.ki0), (1, wt.ki1)):
                        if half is None:
                            continue
                        mi, c, w = half
                        lhsT = wg_tiles[gi][ki * 64:(ki + 1) * 64,
                                            jj * BLK:(jj + 1) * BLK]
                        for th in range(n_th):
                            done[p][(th, ki, mi)] += 1
                            first = done[p][(th, ki, mi)] == 1
                            last = done[p][(th, ki, mi)] == nmm[p][(ki, mi)]
                            nc.tensor.matmul(
                                psum[(p, th)][ki][mi * 64:(mi + 1) * 64, :],
                                lhsT, x_ap(c, ki, th),
                                start=first, stop=last,
                                tile_position=(ki * 64, mi * 64),
                                skip_group_check=True,
                            )

            osb_cur = {}

            def eviction_th(p, th):
                if p not in osb_cur:
                    osb_cur[p] = opool.tile([128, n_th * 512], sdt,
                                            tag="o16", name=f"o{p}")
                osb = osb_cur[p][:, th * 512:(th + 1) * 512]
                pt = psum.pop((p, th))
                if all(v > 0 for v in nmm[p].values()):
                    nc.scalar.activation(
                        osb, pt[0][:],
                        mybir.ActivationFunctionType.Identity,
                        bias=bias_sb[:, p:p + 1], scale=1.0)
                    nc.vector.tensor_add(osb, osb, pt[1][:])
                else:
                    for mi in (0, 1):
                        oh = osb[mi * 64:(mi + 1) * 64, :]
                        bh = bias_sb[mi * 64:(mi + 1) * 64, p:p + 1]
                        srcs = [pt[ki][mi * 64:(mi + 1) * 64, :]
                                for ki in (0, 1) if nmm[p][(ki, mi)] > 0]
                        if not srcs:
                            nc.vector.memset(oh, 0.0)
                            nc.vector.tensor_scalar_add(oh, oh, bh)
                        else:
                            nc.scalar.activation(
                                oh, srcs[0],
                                mybir.ActivationFunctionType.Identity,
                                bias=bh, scale=1.0)
                            if len(srcs) > 1:
                                nc.vector.tensor_add(oh, oh, srcs[1])
                if p >= len(pairs) - 2:
                    nc.sync.dma_start(
                        yt_d[p * 128:(p + 1) * 128,
                             th * 512:(th + 1) * 512], osb)
                    if th == n_th - 1:
                        osb_cur.pop(p)
                elif th == n_th - 1:
                    nc.sync.dma_start(yt_d[p * 128:(p + 1) * 128, :],
                                      osb_cur.pop(p)[:])

            pair_base = []
            acc = 0
            for ps_ in pairs:
                pair_base.append(acc)
                acc += len(ps_.wtiles)

            for p, ps_ in enumerate(pairs):
                if not ps_.wtiles:
                    continue
                emit_pair(p, ps_)
                for th in range(n_th):
                    eviction_th(p, th)

            for p, ps_ in enumerate(pairs):
                if ps_.wtiles:
                    continue
                osb = opool.tile([128, n_th * 512], sdt, tag="o16",
                                 name=f"oz{p}")
                nc.vector.memset(osb[:], 0.0)
                nc.vector.tensor_scalar_add(osb[:], osb[:],
                                            bias_sb[:, p:p + 1])
                nc.sync.dma_start(yt_d[p * 128:(p + 1) * 128, :], osb[:])
    nc.compile()
    return nc


def kernel(x, weight, bias, row_idx, col_idx):
    x = np.asarray(x, dtype=np.float32)
    weight = np.asarray(weight, dtype=np.float32)
    bias = np.asarray(bias, dtype=np.float32)
    row_idx = np.asarray(row_idx)
    col_idx = np.asarray(col_idx)
    ntok_total = x.shape[0]
    assert ntok_total % N_CORES == 0
    ntok = ntok_total // N_CORES

    pairs, n_wt, side_of, tile_of, touched = _make_schedule(row_idx, col_idx)
    wgrp, bias_pk = _pack_host_arrays(weight, bias, pairs, n_wt)
    nc = _build_kernel(pairs, n_wt, side_of, tile_of, ntok)

    # tile-major x^T: tile i rows = [block touched[0][i]; block touched[1][i]]
    perm = np.empty(IN_BLK, dtype=np.int64)
    for i in range(IN_BLK // 2):
        perm[2 * i] = touched[0][i]
        perm[2 * i + 1] = touched[1][i]
    xT = np.ascontiguousarray(x.T).astype(BF16)      # [D_IN, ntok_total]
    xt_full = xT.reshape(IN_BLK, BLK, ntok_total)[perm].reshape(
        D_IN, ntok_total)

    in_maps = []
    for c in range(N_CORES):
        xt = np.ascontiguousarray(xt_full[:, c * ntok:(c + 1) * ntok])
        in_maps.append({"xt": xt, "wgrp": wgrp, "bias_pk": bias_pk})

    res = run_bass_kernel_spmd(nc, in_maps, core_ids=list(range(N_CORES)))

    # un-permute pair-major rows, transpose, upcast
    row_perm = np.empty(D_OUT, dtype=np.int64)
    for p, ps in enumerate(pairs):
        r1, r2 = ps.r
        row_perm[r1 * BLK:(r1 + 1) * BLK] = np.arange(
            p * 128, p * 128 + 64)
        row_perm[r2 * BLK:(r2 + 1) * BLK] = np.arange(
            p * 128 + 64, p * 128 + 128)
    y = np.empty((ntok_total, D_OUT), dtype=np.float32)
    for c in range(N_CORES):
        yt = np.asarray(res.results[c]["yt"])          # [D_OUT, ntok] bf16
        y[c * ntok:(c + 1) * ntok] = yt[row_perm].T.astype(np.float32)
    return y
